# revision 45
# baseline (speedup 1.0000x reference)
"""Additive attention (Bahdanau) fused Trainium2 kernel, data-parallel over batch.

Math: with q = Q @ Wq.T + bq, k = K @ Wk.T + bk,
  scores[b,i,j] = tanh( w_s . (q[b,i] + k[b,j]) + b_s )
                = tanh( qs[b,i] + ks[b,j] + c )
where qs = Q @ (Wq.T @ w_s), ks = K @ (Wk.T @ w_s), c = (bq+bk).w_s + b_s.
The (B,Lq,Lk,H) intermediate is never materialized. tanh bounds scores in
[-1,1], so the softmax needs no max-subtraction; masking is a per-key -120
additive bias on the exp input (exp(-120±1) underflows to exactly 0, matching
the reference's -1e6 fill). The softmax denominator comes from a ones-column
appended to V inside the attn @ V matmul (PSUM accumulates fp32).

Layout on each core (batch element b): scores kept TRANSPOSED [k, q] so the
key dim sits on partitions: the attn @ V contraction over k runs on the PE,
the softmax mask is a per-partition exp bias, and no 512x512 transpose is
ever needed. qs enters via a PE ones-broadcast, ks via the ACT bias port.

Sharding: batch B=8 across 8 NeuronCores, one batch element per core. Inputs
are host-packed into contiguous [128, N] blocks (pure relayout) so each lands
with one simple 2D DMA split over two HWDGE queues (sync + scalar).
"""

from contextlib import ExitStack

import numpy as np

import concourse.tile as tile
from concourse import bacc, mybir
from concourse.bass import ts
from concourse.bass_utils import run_bass_kernel_spmd
from concourse.masks import make_identity

B, LQ, LK = 8, 512, 512
F = 256          # feature dim of Q/K/V
H = 128          # hidden dim of the additive-attention MLP
P = 128          # SBUF partitions
QT = LQ // P     # query chunks per core
KT = LK // P     # key chunks per core
NCORES = 8
MASK_BIAS = -120.0  # exp(-120 + [-1,1]) == 0.0 in fp32

F32 = mybir.dt.float32
BF16 = mybir.dt.bfloat16

# wpack column layout: Wq | Wk | ws_bcast(128) | ws | bq | bk | vl | iota4 | bs
WQ0, WK0 = 0, F
WSB0 = 2 * F                      # ws broadcast block [128h, 128] (col-replicated)
WS_C = WSB0 + P                   # ws as a single column
BQ_C, BK_C, VL_C = WS_C + 1, WS_C + 2, WS_C + 3
IOTA0 = WS_C + 4
BS_C = IOTA0 + KT
WPACK_W = BS_C + 1

TRACE = False
LAST_RESULT = None


def _emit(tc, d):
    nc = tc.nc
    X = mybir.AxisListType
    A = mybir.AluOpType
    AF = mybir.ActivationFunctionType

    with ExitStack() as ctx:
        consts = ctx.enter_context(tc.tile_pool(name="consts", bufs=1))
        big = ctx.enter_context(tc.tile_pool(name="big", bufs=1))
        small = ctx.enter_context(tc.tile_pool(name="small", bufs=1))
        st_pool = ctx.enter_context(tc.tile_pool(name="st", bufs=2))
        ps_uv = ctx.enter_context(tc.tile_pool(name="ps_uv", bufs=1, space="PSUM"))
        ps_c = ctx.enter_context(tc.tile_pool(name="ps_c", bufs=1, space="PSUM"))
        ps_bc = ctx.enter_context(tc.tile_pool(name="ps_bc", bufs=1, space="PSUM"))
        ps_acc = ctx.enter_context(tc.tile_pool(name="ps_acc", bufs=1, space="PSUM"))

        # ---- DMA issue: sync [wpack, k halves, out], scalar [q halves, vaug]
        # q/k in half-tiles: each half has its OWN tile so compute on the
        # first half never waits on the second (Tile deps are per-tile).
        wpack = consts.tile([P, WPACK_W], F32)
        nc.sync.dma_start(wpack, d["wpack"])
        H2 = QT // 2
        qd = d["qpk"].rearrange("p (t f) -> p t f", f=F)
        kd = d["kpk"].rearrange("p (t f) -> p t f", f=F)
        q_h = [big.tile([P, H2, F], F32, name=f"qh{i}") for i in range(2)]
        k_h = [big.tile([P, H2, F], F32, name=f"kh{i}") for i in range(2)]
        nc.scalar.dma_start(q_h[0], qd[:, 0:H2])
        nc.sync.dma_start(k_h[0], kd[:, 0:H2])
        nc.scalar.dma_start(q_h[1], qd[:, H2:QT])
        nc.sync.dma_start(k_h[1], kd[:, H2:QT])
        aug_f = big.tile([P, KT * (F + 1)], F32)
        nc.scalar.dma_start(aug_f.rearrange("p (t g) -> p t g", g=F + 1),
                            d["vaug"].rearrange("p (t g) -> p t g", g=F + 1))
        q_ch = [q_h[t // H2][:, t % H2, :] for t in range(QT)]
        k_ch = [k_h[t // H2][:, t % H2, :] for t in range(KT)]

        # ACT table prefetch: dummy Exp loads exp_and_others during the DMAs
        warm = consts.tile([1, 1], F32)
        nc.vector.memset(warm, 0.0)
        nc.scalar.activation(warm, warm, AF.Exp)

        id128 = consts.tile([P, P], F32)
        make_identity(nc, id128)

        # ---- weight prep (all off the q/k critical path) ----
        # One matmul gives [u|v] broadcast to all partitions:
        # uv_bc[p, f] = sum_h ws[h] * W[h, f]  (bf16 operands: 4x col rate)
        wf_bf = consts.tile([P, BK_C + 1], BF16)
        nc.vector.tensor_copy(wf_bf, wpack[:, 0:BK_C + 1])
        uv_bc_ps = ps_uv.tile([P, 2 * F], F32)
        nc.tensor.matmul(uv_bc_ps, wf_bf[:, WSB0:WSB0 + P], wf_bf[:, 0:2 * F],
                         start=True, stop=True)

        # expbias[p, c] = (p + 128c >= valid_len) ? MASK_BIAS : 0
        expbias = small.tile([P, KT], F32)
        nc.vector.tensor_scalar(expbias, wpack[:, IOTA0:IOTA0 + KT],
                                wpack[:, VL_C:VL_C + 1], MASK_BIAS, A.is_ge, A.mult)

        # c_bc[p] = ws.(bq+bk) broadcast: two tiny PE matmuls against the
        # ws broadcast block; bs arrives host-broadcast and folds in later.
        c_bc_ps = ps_c.tile([P, 1], F32)
        nc.tensor.matmul(c_bc_ps, wf_bf[:, WSB0:WSB0 + P],
                         wf_bf[:, BQ_C:BQ_C + 1], start=True, stop=False)
        nc.tensor.matmul(c_bc_ps, wf_bf[:, WSB0:WSB0 + P],
                         wf_bf[:, BK_C:BK_C + 1], start=False, stop=True)



        # ---- qs matvec on DVE; PE transpose per chunk into one [1, LQ] row ----
        # Per chunk: qm = q*u (DVE), reduce to qs column (DVE), then ONE
        # matmul fuses transpose+broadcast: lhsT = qs column replicated over
        # the free dim (stride-0), rhs = identity ->
        # out[p, j] = sum_q qs[q] * I[q, j] broadcast to all 128 partitions.
        qs_bc_ps = ps_bc.tile([P, LQ], F32)
        last_rq = None
        for t in range(QT):
            qm = st_pool.tile([P, F], BF16, tag="qm", bufs=4)
            nc.vector.tensor_tensor(qm, q_ch[t], uv_bc_ps[:, 0:F], A.mult)
            qs_col = small.tile([P, 1], F32, name=f"qs_col{t}")
            last_rq = nc.vector.reduce_sum(qs_col, qm, axis=X.X)
            nc.tensor.matmul(qs_bc_ps[:, ts(t, P)],
                             qs_col.broadcast_to([P, P]), id128,
                             start=True, stop=True)

        # ks matvec all on DVE (GpSimd's semaphore-release latency is 2-4us
        # and unpredictable); ordering edge keeps ks behind qs in the FIFO
        ks_cols = []
        for t in range(KT):
            km = st_pool.tile([P, F], BF16, tag="km", bufs=4)
            mk = nc.vector.tensor_tensor(km, k_ch[t], uv_bc_ps[:, F:2 * F],
                                         A.mult)
            if t == 0:
                tile.add_dep_helper(mk.ins, last_rq.ins, sync=False,
                                    reason="qs before ks on DVE")
            ks_raw = small.tile([P, 1], F32, name=f"ks_raw{t}")
            nc.vector.reduce_sum(ks_raw, km, axis=X.X)
            ksc = small.tile([P, 1], F32, name=f"ksc{t}")
            nc.vector.tensor_scalar(ksc, ks_raw, c_bc_ps,
                                    wpack[:, BS_C:BS_C + 1], A.add, A.add)
            ks_cols.append(ksc)

        # aug -> bf16 per chunk on GpSimd (idle otherwise); per-chunk tiles
        # so the first attn@V matmul doesn't wait on later casts
        aug_bf = [big.tile([P, F + 1], BF16, name=f"aug_bf{c}") for c in range(KT)]
        for c in range(KT):
            nc.gpsimd.tensor_copy(aug_bf[c],
                                  aug_f[:, c * (F + 1):(c + 1) * (F + 1)])

        # ---- scores.T -> exp (bf16) -> attn.T @ [V | 1] ----
        accs = [ps_acc.tile([P, F + 1], F32, tag=f"acc{qc}", name=f"acc{qc}")
                for qc in range(QT)]
        for c in range(KT):
            sT = st_pool.tile([P, LQ], F32, tag="sT")
            nc.scalar.activation(sT, qs_bc_ps, AF.Tanh, bias=ks_cols[c])
            eT = st_pool.tile([P, LQ], BF16, tag="eT", bufs=3)
            nc.scalar.activation(eT, sT, AF.Exp, bias=expbias[:, c:c + 1])
            for qc in range(QT):
                nc.tensor.matmul(accs[qc], eT[:, ts(qc, P)], aug_bf[c],
                                 start=(c == 0), stop=(c == KT - 1))

        # ---- normalize (split ACT/DVE), store in halves (own tiles) ----
        out_h = [big.tile([P, 2 * F], F32, name=f"outh{i}") for i in range(2)]
        for qc in range(QT):
            o_view = out_h[qc // 2][:, ts(qc % 2, F)]
            rec = small.tile([P, 1], F32, tag=f"rec{qc}", name=f"rec{qc}")
            nc.vector.reciprocal(rec, accs[qc][:, F:F + 1])
            if qc % 2 == 0:
                nc.scalar.activation(o_view, accs[qc][:, 0:F],
                                     AF.Copy, bias=0.0, scale=rec)
            else:
                nc.vector.tensor_scalar(o_view, accs[qc][:, 0:F],
                                        rec, None, A.mult)
            if qc == 1:
                nc.sync.dma_start(d["out"][:, 0:2 * F], out_h[0])
        nc.sync.dma_start(d["out"][:, 2 * F:4 * F], out_h[1])


_NC = None


def _build_nc():
    nc = bacc.Bacc("TRN2", target_bir_lowering=False, debug=False, num_devices=1)
    d = {}
    d["wpack"] = nc.dram_tensor("wpack", [P, WPACK_W], F32, kind="ExternalInput").ap()
    d["qpk"] = nc.dram_tensor("qpk", [P, QT * F], F32, kind="ExternalInput").ap()
    d["kpk"] = nc.dram_tensor("kpk", [P, KT * F], F32, kind="ExternalInput").ap()
    d["vaug"] = nc.dram_tensor("vaug", [P, KT * (F + 1)], F32, kind="ExternalInput").ap()
    d["out"] = nc.dram_tensor("out", [P, QT * F], F32, kind="ExternalOutput").ap()

    with tile.TileContext(nc) as tc:
        _emit(tc, d)
    nc.compile()
    return nc


def get_nc():
    global _NC
    if _NC is None:
        _NC = _build_nc()
    return _NC


def make_in_maps(queries, keys, values, valid_lens, Wq, bq, Wk, bk, w_s, b_s):
    f32 = lambda a: np.asarray(a, dtype=np.float32)
    qs, ks, vs = f32(queries), f32(keys), f32(values)
    vl = np.asarray(valid_lens)

    wpack = np.zeros((P, WPACK_W), np.float32)
    wpack[:, WQ0:WQ0 + F] = f32(Wq)
    wpack[:, WK0:WK0 + F] = f32(Wk)
    wpack[:, WSB0:WSB0 + P] = f32(w_s)[:, None]          # ws broadcast block
    wpack[:, WS_C] = f32(w_s)
    wpack[:, BQ_C] = f32(bq)
    wpack[:, BK_C] = f32(bk)
    wpack[:, IOTA0:IOTA0 + KT] = (np.arange(P, dtype=np.float32)[:, None]
                                  + P * np.arange(KT, dtype=np.float32)[None, :])
    wpack[:, BS_C] = f32(b_s).reshape(-1)[0]

    # [L, F] -> [P, T*F] with col t*F+f <-> row t*P+p  (pure relayout)
    def pack_tiles(a):  # a: [L, F]
        t = a.reshape(-1, P, F)                    # [T, P, F]
        return np.ascontiguousarray(t.transpose(1, 0, 2).reshape(P, -1))

    in_maps = []
    for b in range(NCORES):
        m = {}
        wp = wpack.copy()
        wp[:, VL_C] = float(vl[b])
        m["wpack"] = wp
        m["qpk"] = pack_tiles(qs[b])
        m["kpk"] = pack_tiles(ks[b])
        va = np.ones((KT, P, F + 1), np.float32)
        va[:, :, :F] = vs[b].reshape(KT, P, F)
        m["vaug"] = np.ascontiguousarray(va.transpose(1, 0, 2).reshape(P, -1))
        in_maps.append(m)
    return in_maps


def kernel(queries, keys, values, valid_lens, Wq, bq, Wk, bk, w_s, b_s):
    global LAST_RESULT
    nc = get_nc()
    in_maps = make_in_maps(queries, keys, values, valid_lens, Wq, bq, Wk, bk, w_s, b_s)
    res = run_bass_kernel_spmd(nc, in_maps, list(range(NCORES)), trace=TRACE)
    LAST_RESULT = res
    out = np.stack([res.results[b]["out"] for b in range(NCORES)], axis=0)
    # [P, T*F] -> [LQ, F]
    out = out.reshape(B, P, QT, F).transpose(0, 2, 1, 3).reshape(B, LQ, F)
    return np.ascontiguousarray(out)


# revision 51
# speedup vs baseline: 1.0697x; 1.0697x over previous
"""Additive attention (Bahdanau) fused Trainium2 kernel, data-parallel over batch.

Math: with q = Q @ Wq.T + bq, k = K @ Wk.T + bk,
  scores[b,i,j] = tanh( w_s . (q[b,i] + k[b,j]) + b_s )
                = tanh( qs[b,i] + ks[b,j] + c )
where qs = Q @ (Wq.T @ w_s), ks = K @ (Wk.T @ w_s), c = (bq+bk).w_s + b_s.
The (B,Lq,Lk,H) intermediate is never materialized. tanh bounds scores in
[-1,1], so the softmax needs no max-subtraction; masking is a per-key -120
additive bias on the exp input (exp(-120±1) underflows to exactly 0, matching
the reference's -1e6 fill). The softmax denominator comes from a ones-column
appended to V inside the attn @ V matmul (PSUM accumulates fp32).

Layout on each core (batch element b): scores kept TRANSPOSED [k, q] so the
key dim sits on partitions: the attn @ V contraction over k runs on the PE,
the softmax mask is a per-partition exp bias, and no 512x512 transpose is
ever needed. qs enters via a PE ones-broadcast, ks via the ACT bias port.

Sharding: batch B=8 across 8 NeuronCores, one batch element per core. Inputs
are host-packed into contiguous [128, N] blocks (pure relayout) so each lands
with one simple 2D DMA split over two HWDGE queues (sync + scalar).
"""

from contextlib import ExitStack

import numpy as np

import concourse.tile as tile
from concourse import bacc, mybir
from concourse.bass import ts
from concourse.bass_utils import run_bass_kernel_spmd
from concourse.masks import make_identity

B, LQ, LK = 8, 512, 512
F = 256          # feature dim of Q/K/V
H = 128          # hidden dim of the additive-attention MLP
P = 128          # SBUF partitions
QT = LQ // P     # query chunks per core
KT = LK // P     # key chunks per core
NCORES = 8
MASK_BIAS = -120.0  # exp(-120 + [-1,1]) == 0.0 in fp32

F32 = mybir.dt.float32
BF16 = mybir.dt.bfloat16

# wpack column layout: Wq | Wk | ws_bcast(128) | ws | bq | bk | vl | iota4 | bs
WQ0, WK0 = 0, F
WSB0 = 2 * F                      # ws broadcast block [128h, 128] (col-replicated)
WS_C = WSB0 + P                   # ws as a single column
BQ_C, BK_C, VL_C = WS_C + 1, WS_C + 2, WS_C + 3
IOTA0 = WS_C + 4
BS_C = IOTA0 + KT
WPACK_W = BS_C + 1

TRACE = False
LAST_RESULT = None


def _emit(tc, d):
    nc = tc.nc
    X = mybir.AxisListType
    A = mybir.AluOpType
    AF = mybir.ActivationFunctionType

    with ExitStack() as ctx:
        consts = ctx.enter_context(tc.tile_pool(name="consts", bufs=1))
        big = ctx.enter_context(tc.tile_pool(name="big", bufs=1))
        small = ctx.enter_context(tc.tile_pool(name="small", bufs=1))
        st_pool = ctx.enter_context(tc.tile_pool(name="st", bufs=2))
        ps_uv = ctx.enter_context(tc.tile_pool(name="ps_uv", bufs=1, space="PSUM"))
        ps_warm = ctx.enter_context(tc.tile_pool(name="ps_warm", bufs=1, space="PSUM"))
        ps_c = ctx.enter_context(tc.tile_pool(name="ps_c", bufs=1, space="PSUM"))
        ps_bc = ctx.enter_context(tc.tile_pool(name="ps_bc", bufs=1, space="PSUM"))
        ps_acc = ctx.enter_context(tc.tile_pool(name="ps_acc", bufs=1, space="PSUM"))

        # ---- DMA issue: sync [wpack, k halves, out], scalar [q halves, vaug]
        # q/k in half-tiles: each half has its OWN tile so compute on the
        # first half never waits on the second (Tile deps are per-tile).
        wpack = consts.tile([P, WPACK_W], F32)
        nc.sync.dma_start(wpack, d["wpack"])
        H2 = QT // 2
        qd = d["qpk"].rearrange("p (t f) -> p t f", f=F)
        kd = d["kpk"].rearrange("p (t f) -> p t f", f=F)
        q_h = [big.tile([P, H2, F], F32, name=f"qh{i}") for i in range(2)]
        k_h = [big.tile([P, H2, F], F32, name=f"kh{i}") for i in range(2)]
        nc.scalar.dma_start(q_h[0], qd[:, 0:H2])
        nc.sync.dma_start(k_h[0], kd[:, 0:H2])
        nc.scalar.dma_start(q_h[1], qd[:, H2:QT])
        nc.sync.dma_start(k_h[1], kd[:, H2:QT])
        aug_f = big.tile([P, KT * (F + 1)], F32)
        nc.scalar.dma_start(aug_f.rearrange("p (t g) -> p t g", g=F + 1),
                            d["vaug"].rearrange("p (t g) -> p t g", g=F + 1))
        q_ch = [q_h[t // H2][:, t % H2, :] for t in range(QT)]
        k_ch = [k_h[t // H2][:, t % H2, :] for t in range(KT)]

        # ACT table prefetch: dummy Exp loads exp_and_others during the DMAs
        warm = consts.tile([1, 1], F32)
        nc.vector.memset(warm, 0.0)
        nc.scalar.activation(warm, warm, AF.Exp)

        id128 = consts.tile([P, P], F32)
        make_identity(nc, id128)

        # ---- weight prep (all off the q/k critical path) ----
        # One matmul gives [u|v] broadcast to all partitions:
        # uv_bc[p, f] = sum_h ws[h] * W[h, f]  (bf16 operands: 4x col rate)
        wf_bf = consts.tile([P, BK_C + 1], BF16)
        nc.vector.tensor_copy(wf_bf, wpack[:, 0:BK_C + 1])
        uv_bc_ps = ps_uv.tile([P, 2 * F], F32)
        nc.tensor.matmul(uv_bc_ps, wf_bf[:, WSB0:WSB0 + P], wf_bf[:, 0:2 * F],
                         start=True, stop=True)

        # expbias[p, c] = (p + 128c >= valid_len) ? MASK_BIAS : 0
        expbias = small.tile([P, KT], F32)
        nc.vector.tensor_scalar(expbias, wpack[:, IOTA0:IOTA0 + KT],
                                wpack[:, VL_C:VL_C + 1], MASK_BIAS, A.is_ge, A.mult)

        # PE warm-up: dense dummy matmuls so the HAM clock gate opens before
        # the transpose-broadcast and attn@V matmuls run (cold PE = 1.2 GHz)
        warm_ps = ps_warm.tile([P, 2 * F], F32)
        for i in range(4):
            nc.tensor.matmul(warm_ps, wf_bf[:, 0:P], wf_bf[:, 0:2 * F],
                             start=True, stop=True)

        # c_bc[p] = ws.(bq+bk) broadcast: two tiny PE matmuls against the
        # ws broadcast block; bs arrives host-broadcast and folds in later.
        c_bc_ps = ps_c.tile([P, 1], F32)
        nc.tensor.matmul(c_bc_ps, wf_bf[:, WSB0:WSB0 + P],
                         wf_bf[:, BQ_C:BQ_C + 1], start=True, stop=False)
        nc.tensor.matmul(c_bc_ps, wf_bf[:, WSB0:WSB0 + P],
                         wf_bf[:, BK_C:BK_C + 1], start=False, stop=True)



        # ---- qs matvec on DVE; PE transpose per chunk into one [1, LQ] row ----
        # Per chunk: qm = q*u (DVE), reduce to qs column (DVE), then ONE
        # matmul fuses transpose+broadcast: lhsT = qs column replicated over
        # the free dim (stride-0), rhs = identity ->
        # out[p, j] = sum_q qs[q] * I[q, j] broadcast to all 128 partitions.
        # All 8 elementwise multiplies on DVE (interleaved q/k); all reduces
        # on ACT via activation accum_out (free row-sum); c/bs folds on DVE.
        qms, kms = [], []
        mult_order = [("q", 0), ("q", 1), ("k", 0), ("q", 2), ("q", 3),
                      ("k", 1), ("k", 2), ("k", 3)]
        for kind, t in mult_order:
            m = st_pool.tile([P, F], BF16, tag=f"{kind}m", bufs=4,
                             name=f"{kind}m{t}")
            if kind == "q":
                nc.vector.tensor_tensor(m, q_ch[t], uv_bc_ps[:, 0:F], A.mult)
                qms.append(m)
            else:
                nc.vector.tensor_tensor(m, k_ch[t], uv_bc_ps[:, F:2 * F], A.mult)
                kms.append(m)

        qs_bc_ps = ps_bc.tile([P, LQ], F32)
        qs_cols, ks_raws = [], []
        red_order = [("q", 0), ("q", 1), ("k", 0), ("q", 2), ("q", 3),
                     ("k", 1), ("k", 2), ("k", 3)]
        for kind, t in red_order:
            col = small.tile([P, 1], F32, name=f"{kind}s_col{t}")
            trash = st_pool.tile([P, F], F32, tag="trash", bufs=2)
            src = qms[t] if kind == "q" else kms[t]
            nc.scalar.activation(trash, src, AF.Identity, accum_out=col)
            if kind == "q":
                qs_cols.append(col)
                nc.tensor.matmul(qs_bc_ps[:, ts(t, P)],
                                 col.broadcast_to([P, P]), id128,
                                 start=True, stop=True)
            else:
                ks_raws.append(col)
        ks_cols = []
        for t in range(KT):
            ksc = small.tile([P, 1], F32, name=f"ksc{t}")
            nc.vector.tensor_scalar(ksc, ks_raws[t], c_bc_ps,
                                    wpack[:, BS_C:BS_C + 1], A.add, A.add)
            ks_cols.append(ksc)

        # aug -> bf16 per chunk on GpSimd (idle otherwise); per-chunk tiles
        # so the first attn@V matmul doesn't wait on later casts
        aug_bf = [big.tile([P, F + 1], BF16, name=f"aug_bf{c}") for c in range(KT)]
        for c in range(KT):
            nc.gpsimd.tensor_copy(aug_bf[c],
                                  aug_f[:, c * (F + 1):(c + 1) * (F + 1)])

        # ---- scores.T -> exp (bf16) -> attn.T @ [V | 1] ----
        accs = [ps_acc.tile([P, F + 1], F32, tag=f"acc{qc}", name=f"acc{qc}")
                for qc in range(QT)]
        for c in range(KT):
            sT = st_pool.tile([P, LQ], F32, tag="sT")
            nc.scalar.activation(sT, qs_bc_ps, AF.Tanh, bias=ks_cols[c])
            eT = st_pool.tile([P, LQ], BF16, tag="eT", bufs=3)
            nc.scalar.activation(eT, sT, AF.Exp, bias=expbias[:, c:c + 1])
            for qc in range(QT):
                nc.tensor.matmul(accs[qc], eT[:, ts(qc, P)], aug_bf[c],
                                 start=(c == 0), stop=(c == KT - 1))

        # ---- normalize (split ACT/DVE), store per-chunk on both queues ----
        for qc in range(QT):
            o_sb = big.tile([P, F], F32, name=f"out{qc}")
            rec = small.tile([P, 1], F32, tag=f"rec{qc}", name=f"rec{qc}")
            nc.vector.reciprocal(rec, accs[qc][:, F:F + 1])
            if qc % 2 == 0:
                nc.scalar.activation(o_sb, accs[qc][:, 0:F],
                                     AF.Copy, bias=0.0, scale=rec)
                nc.sync.dma_start(d["out"][:, ts(qc, F)], o_sb)
            else:
                nc.vector.tensor_scalar(o_sb, accs[qc][:, 0:F],
                                        rec, None, A.mult)
                nc.scalar.dma_start(d["out"][:, ts(qc, F)], o_sb)

        # keep the PE warm-up psum alive/read so it isn't dead-code removed
        warm_junk = small.tile([P, 1], F32)
        nc.vector.reduce_sum(warm_junk, warm_ps[:, 0:8], axis=X.X)


_NC = None


def _build_nc():
    nc = bacc.Bacc("TRN2", target_bir_lowering=False, debug=False, num_devices=1)
    d = {}
    d["wpack"] = nc.dram_tensor("wpack", [P, WPACK_W], F32, kind="ExternalInput").ap()
    d["qpk"] = nc.dram_tensor("qpk", [P, QT * F], F32, kind="ExternalInput").ap()
    d["kpk"] = nc.dram_tensor("kpk", [P, KT * F], F32, kind="ExternalInput").ap()
    d["vaug"] = nc.dram_tensor("vaug", [P, KT * (F + 1)], F32, kind="ExternalInput").ap()
    d["out"] = nc.dram_tensor("out", [P, QT * F], F32, kind="ExternalOutput").ap()

    with tile.TileContext(nc) as tc:
        _emit(tc, d)
    nc.compile()
    return nc


def get_nc():
    global _NC
    if _NC is None:
        _NC = _build_nc()
    return _NC


def make_in_maps(queries, keys, values, valid_lens, Wq, bq, Wk, bk, w_s, b_s):
    f32 = lambda a: np.asarray(a, dtype=np.float32)
    qs, ks, vs = f32(queries), f32(keys), f32(values)
    vl = np.asarray(valid_lens)

    wpack = np.zeros((P, WPACK_W), np.float32)
    wpack[:, WQ0:WQ0 + F] = f32(Wq)
    wpack[:, WK0:WK0 + F] = f32(Wk)
    wpack[:, WSB0:WSB0 + P] = f32(w_s)[:, None]          # ws broadcast block
    wpack[:, WS_C] = f32(w_s)
    wpack[:, BQ_C] = f32(bq)
    wpack[:, BK_C] = f32(bk)
    wpack[:, IOTA0:IOTA0 + KT] = (np.arange(P, dtype=np.float32)[:, None]
                                  + P * np.arange(KT, dtype=np.float32)[None, :])
    wpack[:, BS_C] = f32(b_s).reshape(-1)[0]

    # [L, F] -> [P, T*F] with col t*F+f <-> row t*P+p  (pure relayout)
    def pack_tiles(a):  # a: [L, F]
        t = a.reshape(-1, P, F)                    # [T, P, F]
        return np.ascontiguousarray(t.transpose(1, 0, 2).reshape(P, -1))

    in_maps = []
    for b in range(NCORES):
        m = {}
        wp = wpack.copy()
        wp[:, VL_C] = float(vl[b])
        m["wpack"] = wp
        m["qpk"] = pack_tiles(qs[b])
        m["kpk"] = pack_tiles(ks[b])
        va = np.ones((KT, P, F + 1), np.float32)
        va[:, :, :F] = vs[b].reshape(KT, P, F)
        m["vaug"] = np.ascontiguousarray(va.transpose(1, 0, 2).reshape(P, -1))
        in_maps.append(m)
    return in_maps


def kernel(queries, keys, values, valid_lens, Wq, bq, Wk, bk, w_s, b_s):
    global LAST_RESULT
    nc = get_nc()
    in_maps = make_in_maps(queries, keys, values, valid_lens, Wq, bq, Wk, bk, w_s, b_s)
    res = run_bass_kernel_spmd(nc, in_maps, list(range(NCORES)), trace=TRACE)
    LAST_RESULT = res
    out = np.stack([res.results[b]["out"] for b in range(NCORES)], axis=0)
    # [P, T*F] -> [LQ, F]
    out = out.reshape(B, P, QT, F).transpose(0, 2, 1, 3).reshape(B, LQ, F)
    return np.ascontiguousarray(out)


# revision 54
# speedup vs baseline: 1.0753x; 1.0052x over previous
"""Additive attention (Bahdanau) fused Trainium2 kernel, data-parallel over batch.

Math: with q = Q @ Wq.T + bq, k = K @ Wk.T + bk,
  scores[b,i,j] = tanh( w_s . (q[b,i] + k[b,j]) + b_s )
                = tanh( qs[b,i] + ks[b,j] + c )
where qs = Q @ (Wq.T @ w_s), ks = K @ (Wk.T @ w_s), c = (bq+bk).w_s + b_s.
The (B,Lq,Lk,H) intermediate is never materialized. tanh bounds scores in
[-1,1], so the softmax needs no max-subtraction; masking is a per-key -120
additive bias on the exp input (exp(-120±1) underflows to exactly 0, matching
the reference's -1e6 fill). The softmax denominator comes from a ones-column
appended to V inside the attn @ V matmul (PSUM accumulates fp32).

Layout on each core (batch element b): scores kept TRANSPOSED [k, q] so the
key dim sits on partitions: the attn @ V contraction over k runs on the PE,
the softmax mask is a per-partition exp bias, and no 512x512 transpose is
ever needed. qs enters via a PE ones-broadcast, ks via the ACT bias port.

Sharding: batch B=8 across 8 NeuronCores, one batch element per core. Inputs
are host-packed into contiguous [128, N] blocks (pure relayout) so each lands
with one simple 2D DMA split over two HWDGE queues (sync + scalar).
"""

from contextlib import ExitStack

import numpy as np

import concourse.tile as tile
from concourse import bacc, mybir
from concourse.bass import ts
from concourse.bass_utils import run_bass_kernel_spmd
from concourse.masks import make_identity

B, LQ, LK = 8, 512, 512
F = 256          # feature dim of Q/K/V
H = 128          # hidden dim of the additive-attention MLP
P = 128          # SBUF partitions
QT = LQ // P     # query chunks per core
KT = LK // P     # key chunks per core
NCORES = 8
MASK_BIAS = -120.0  # exp(-120 + [-1,1]) == 0.0 in fp32

F32 = mybir.dt.float32
BF16 = mybir.dt.bfloat16

# wpack column layout: Wq | Wk | ws_bcast(128) | ws | bq | bk | vl | iota4 | bs
WQ0, WK0 = 0, F
WSB0 = 2 * F                      # ws broadcast block [128h, 128] (col-replicated)
WS_C = WSB0 + P                   # ws as a single column
BQ_C, BK_C, VL_C = WS_C + 1, WS_C + 2, WS_C + 3
IOTA0 = WS_C + 4
BS_C = IOTA0 + KT
WPACK_W = BS_C + 1

TRACE = False
LAST_RESULT = None


def _emit(tc, d):
    nc = tc.nc
    X = mybir.AxisListType
    A = mybir.AluOpType
    AF = mybir.ActivationFunctionType

    with ExitStack() as ctx:
        consts = ctx.enter_context(tc.tile_pool(name="consts", bufs=1))
        big = ctx.enter_context(tc.tile_pool(name="big", bufs=1))
        small = ctx.enter_context(tc.tile_pool(name="small", bufs=1))
        st_pool = ctx.enter_context(tc.tile_pool(name="st", bufs=2))
        ps_uv = ctx.enter_context(tc.tile_pool(name="ps_uv", bufs=1, space="PSUM"))
        ps_warm = ctx.enter_context(tc.tile_pool(name="ps_warm", bufs=1, space="PSUM"))
        ps_c = ctx.enter_context(tc.tile_pool(name="ps_c", bufs=1, space="PSUM"))
        ps_bc = ctx.enter_context(tc.tile_pool(name="ps_bc", bufs=1, space="PSUM"))
        ps_acc = ctx.enter_context(tc.tile_pool(name="ps_acc", bufs=1, space="PSUM"))

        # ---- DMA issue: sync [wpack, k halves, out], scalar [q halves, vaug]
        # q/k in half-tiles: each half has its OWN tile so compute on the
        # first half never waits on the second (Tile deps are per-tile).
        wpack = consts.tile([P, WPACK_W], F32)
        nc.sync.dma_start(wpack, d["wpack"])
        H2 = QT // 2
        qd = d["qpk"].rearrange("p (t f) -> p t f", f=F)
        kd = d["kpk"].rearrange("p (t f) -> p t f", f=F)
        q_h = [big.tile([P, H2, F], F32, name=f"qh{i}") for i in range(2)]
        k_h = [big.tile([P, H2, F], F32, name=f"kh{i}") for i in range(2)]
        nc.scalar.dma_start(q_h[0], qd[:, 0:H2])
        nc.sync.dma_start(k_h[0], kd[:, 0:H2])
        nc.scalar.dma_start(q_h[1], qd[:, H2:QT])
        nc.sync.dma_start(k_h[1], kd[:, H2:QT])
        aug_f = big.tile([P, KT * (F + 1)], F32)
        nc.scalar.dma_start(aug_f.rearrange("p (t g) -> p t g", g=F + 1),
                            d["vaug"].rearrange("p (t g) -> p t g", g=F + 1))
        q_ch = [q_h[t // H2][:, t % H2, :] for t in range(QT)]
        k_ch = [k_h[t // H2][:, t % H2, :] for t in range(KT)]

        # ACT table prefetch: dummy Exp loads exp_and_others during the DMAs
        warm = consts.tile([1, 1], F32)
        nc.vector.memset(warm, 0.0)
        nc.scalar.activation(warm, warm, AF.Exp)

        id128 = consts.tile([P, P], F32)
        make_identity(nc, id128)

        # ---- weight prep (all off the q/k critical path) ----
        # One matmul gives [u|v] broadcast to all partitions:
        # uv_bc[p, f] = sum_h ws[h] * W[h, f]  (bf16 operands: 4x col rate)
        wf_bf = consts.tile([P, BK_C + 1], BF16)
        nc.vector.tensor_copy(wf_bf, wpack[:, 0:BK_C + 1])
        uv_bc_ps = ps_uv.tile([P, 2 * F], F32)
        nc.tensor.matmul(uv_bc_ps, wf_bf[:, WSB0:WSB0 + P], wf_bf[:, 0:2 * F],
                         start=True, stop=True)

        # expbias[p, c] = (p + 128c >= valid_len) ? MASK_BIAS : 0
        expbias = small.tile([P, KT], F32)
        nc.vector.tensor_scalar(expbias, wpack[:, IOTA0:IOTA0 + KT],
                                wpack[:, VL_C:VL_C + 1], MASK_BIAS, A.is_ge, A.mult)

        # PE warm-up: dense dummy matmuls so the HAM clock gate opens before
        # the transpose-broadcast and attn@V matmuls run (cold PE = 1.2 GHz)
        warm_ps = ps_warm.tile([P, 2 * F], F32)
        for i in range(4):
            nc.tensor.matmul(warm_ps, wf_bf[:, 0:P], wf_bf[:, 0:2 * F],
                             start=True, stop=True)

        # c_bc[p] = ws.(bq+bk) broadcast: two tiny PE matmuls against the
        # ws broadcast block; bs arrives host-broadcast and folds in later.
        c_bc_ps = ps_c.tile([P, 1], F32)
        nc.tensor.matmul(c_bc_ps, wf_bf[:, WSB0:WSB0 + P],
                         wf_bf[:, BQ_C:BQ_C + 1], start=True, stop=False)
        nc.tensor.matmul(c_bc_ps, wf_bf[:, WSB0:WSB0 + P],
                         wf_bf[:, BK_C:BK_C + 1], start=False, stop=True)



        # ---- qs matvec on DVE; PE transpose per chunk into one [1, LQ] row ----
        # Per chunk: qm = q*u (DVE), reduce to qs column (DVE), then ONE
        # matmul fuses transpose+broadcast: lhsT = qs column replicated over
        # the free dim (stride-0), rhs = identity ->
        # out[p, j] = sum_q qs[q] * I[q, j] broadcast to all 128 partitions.
        # Multiplies on DVE; q reduces on ACT (accum_out, gate the
        # transpose-broadcast matmuls); k reduces + c/bs folds on DVE,
        # emitted right after their multiply so tanh_c is never gated late.
        qs_bc_ps = ps_bc.tile([P, LQ], F32)
        ks_cols = [None] * KT

        def q_step(t):
            qm = st_pool.tile([P, F], BF16, tag="qm", bufs=4, name=f"qm{t}")
            nc.vector.tensor_tensor(qm, q_ch[t], uv_bc_ps[:, 0:F], A.mult)
            col = small.tile([P, 1], F32, name=f"qs_col{t}")
            trash = st_pool.tile([P, F], F32, tag="trash", bufs=2,
                                 name=f"trash{t}")
            nc.scalar.activation(trash, qm, AF.Identity, accum_out=col)
            nc.tensor.matmul(qs_bc_ps[:, ts(t, P)],
                             col.broadcast_to([P, P]), id128,
                             start=True, stop=True)

        def k_step(t):
            km = st_pool.tile([P, F], BF16, tag="km", bufs=4, name=f"km{t}")
            nc.vector.tensor_tensor(km, k_ch[t], uv_bc_ps[:, F:2 * F], A.mult)
            ks_raw = small.tile([P, 1], F32, name=f"ks_raw{t}")
            nc.vector.reduce_sum(ks_raw, km, axis=X.X)
            ksc = small.tile([P, 1], F32, name=f"ksc{t}")
            nc.vector.tensor_scalar(ksc, ks_raw, c_bc_ps,
                                    wpack[:, BS_C:BS_C + 1], A.add, A.add)
            ks_cols[t] = ksc

        q_step(0)
        q_step(1)
        k_step(0)
        q_step(2)
        q_step(3)
        k_step(1)
        k_step(2)
        k_step(3)

        # aug -> bf16 per chunk on GpSimd (idle otherwise); per-chunk tiles
        # so the first attn@V matmul doesn't wait on later casts
        aug_bf = [big.tile([P, F + 1], BF16, name=f"aug_bf{c}") for c in range(KT)]
        for c in range(KT):
            nc.gpsimd.tensor_copy(aug_bf[c],
                                  aug_f[:, c * (F + 1):(c + 1) * (F + 1)])

        # ---- scores.T -> exp (bf16) -> attn.T @ [V | 1] ----
        accs = [ps_acc.tile([P, F + 1], F32, tag=f"acc{qc}", name=f"acc{qc}")
                for qc in range(QT)]
        for c in range(KT):
            sT = st_pool.tile([P, LQ], F32, tag="sT")
            nc.scalar.activation(sT, qs_bc_ps, AF.Tanh, bias=ks_cols[c])
            eT = st_pool.tile([P, LQ], BF16, tag="eT", bufs=3)
            nc.scalar.activation(eT, sT, AF.Exp, bias=expbias[:, c:c + 1])
            for qc in range(QT):
                nc.tensor.matmul(accs[qc], eT[:, ts(qc, P)], aug_bf[c],
                                 start=(c == 0), stop=(c == KT - 1))

        # ---- normalize (split ACT/DVE), store per-chunk on both queues ----
        recs = []
        for qc in range(QT):
            rec = small.tile([P, 1], F32, tag=f"rec{qc}", name=f"rec{qc}")
            nc.vector.reciprocal(rec, accs[qc][:, F:F + 1])
            recs.append(rec)
        for qc in range(QT):
            o_sb = big.tile([P, F], F32, name=f"out{qc}")
            if qc % 2 == 0:
                nc.scalar.activation(o_sb, accs[qc][:, 0:F],
                                     AF.Copy, bias=0.0, scale=recs[qc])
                nc.sync.dma_start(d["out"][:, ts(qc, F)], o_sb)
            else:
                nc.vector.tensor_scalar(o_sb, accs[qc][:, 0:F],
                                        recs[qc], None, A.mult)
                nc.scalar.dma_start(d["out"][:, ts(qc, F)], o_sb)

        # keep the PE warm-up psum alive/read so it isn't dead-code removed
        warm_junk = small.tile([P, 1], F32)
        nc.vector.reduce_sum(warm_junk, warm_ps[:, 0:8], axis=X.X)


_NC = None


def _build_nc():
    nc = bacc.Bacc("TRN2", target_bir_lowering=False, debug=False, num_devices=1)
    d = {}
    d["wpack"] = nc.dram_tensor("wpack", [P, WPACK_W], F32, kind="ExternalInput").ap()
    d["qpk"] = nc.dram_tensor("qpk", [P, QT * F], F32, kind="ExternalInput").ap()
    d["kpk"] = nc.dram_tensor("kpk", [P, KT * F], F32, kind="ExternalInput").ap()
    d["vaug"] = nc.dram_tensor("vaug", [P, KT * (F + 1)], F32, kind="ExternalInput").ap()
    d["out"] = nc.dram_tensor("out", [P, QT * F], F32, kind="ExternalOutput").ap()

    with tile.TileContext(nc) as tc:
        _emit(tc, d)
    nc.compile()
    return nc


def get_nc():
    global _NC
    if _NC is None:
        _NC = _build_nc()
    return _NC


def make_in_maps(queries, keys, values, valid_lens, Wq, bq, Wk, bk, w_s, b_s):
    f32 = lambda a: np.asarray(a, dtype=np.float32)
    qs, ks, vs = f32(queries), f32(keys), f32(values)
    vl = np.asarray(valid_lens)

    wpack = np.zeros((P, WPACK_W), np.float32)
    wpack[:, WQ0:WQ0 + F] = f32(Wq)
    wpack[:, WK0:WK0 + F] = f32(Wk)
    wpack[:, WSB0:WSB0 + P] = f32(w_s)[:, None]          # ws broadcast block
    wpack[:, WS_C] = f32(w_s)
    wpack[:, BQ_C] = f32(bq)
    wpack[:, BK_C] = f32(bk)
    wpack[:, IOTA0:IOTA0 + KT] = (np.arange(P, dtype=np.float32)[:, None]
                                  + P * np.arange(KT, dtype=np.float32)[None, :])
    wpack[:, BS_C] = f32(b_s).reshape(-1)[0]

    # [L, F] -> [P, T*F] with col t*F+f <-> row t*P+p  (pure relayout)
    def pack_tiles(a):  # a: [L, F]
        t = a.reshape(-1, P, F)                    # [T, P, F]
        return np.ascontiguousarray(t.transpose(1, 0, 2).reshape(P, -1))

    in_maps = []
    for b in range(NCORES):
        m = {}
        wp = wpack.copy()
        wp[:, VL_C] = float(vl[b])
        m["wpack"] = wp
        m["qpk"] = pack_tiles(qs[b])
        m["kpk"] = pack_tiles(ks[b])
        va = np.ones((KT, P, F + 1), np.float32)
        va[:, :, :F] = vs[b].reshape(KT, P, F)
        m["vaug"] = np.ascontiguousarray(va.transpose(1, 0, 2).reshape(P, -1))
        in_maps.append(m)
    return in_maps


def kernel(queries, keys, values, valid_lens, Wq, bq, Wk, bk, w_s, b_s):
    global LAST_RESULT
    nc = get_nc()
    in_maps = make_in_maps(queries, keys, values, valid_lens, Wq, bq, Wk, bk, w_s, b_s)
    res = run_bass_kernel_spmd(nc, in_maps, list(range(NCORES)), trace=TRACE)
    LAST_RESULT = res
    out = np.stack([res.results[b]["out"] for b in range(NCORES)], axis=0)
    # [P, T*F] -> [LQ, F]
    out = out.reshape(B, P, QT, F).transpose(0, 2, 1, 3).reshape(B, LQ, F)
    return np.ascontiguousarray(out)
